# revision 5
# baseline (speedup 1.0000x reference)
"""RBF (Gaussian) kernel matrix on 8 TRN2 NeuronCores.

out[i, j] = exp(-gamma * ||x_i - y_j||^2),  x: [8192, 64], y: [8192, 64].

Strategy: shard rows of x across 8 cores (each computes a [1024, 8192]
tile), replicate y.  The squared distance is produced directly by matmul
via augmented vectors:

    u_i = [-2*x_i, |x_i|^2, 1]   (67 live rows)
    v_j = [   y_j,       1, |y_j|^2]

so  u_i . v_j = |x_i|^2 + |y_j|^2 - 2 x_i.y_j = dist2[i, j].  PSUM then
holds dist2 directly and one ScalarE activation computes
exp(-gamma * dist2) per [128, 2048] tile — no vector-engine work at all.

The augmented operands are zero-padded from 67 to 128 partitions: DMA
loads of <128-partition tiles degenerate to serial single-engine
descriptors (~27 GB/s), while full 128-partition loads spray across all
16 SDMA engines (~400 GB/s).  The extra zero rows are free for the
matmul (stream length is set by the moving operand's free dim, not K).

Output is staged into [128, 8192] SBUF strips; each strip is stored
with a single 4 MB DMA (32 KB per partition) for near-peak HBM write
bandwidth.
"""

import numpy as np

N_X, N_Y, D = 8192, 8192, 64
N_CORES = 8
N_PER = N_X // N_CORES  # rows of x per core
K_LIVE = D + 3  # 67: [-2x, x2-2D, 1, 1] . [y, 1, y2-2D, 2D]
KP = 128  # padded partition dim for fast (16-engine) DMA loads

# Filled by kernel() with the BassKernelResults of the last run
# (test.py reads exec_time_ns from here when BASS_TRACE=1).
LAST_RESULTS = None

_BUILD_CACHE = {}


def _build(gamma: float, n_per: int, m_tot: int):
    """Build + compile the single-core Bass program (same on all cores)."""
    import concourse.bacc as bacc
    import concourse.mybir as mybir
    import concourse.tile as tile

    key = (gamma, n_per, m_tot)
    if key in _BUILD_CACHE:
        return _BUILD_CACHE[key]

    dt = mybir.dt
    nc = bacc.Bacc("TRN2", target_bir_lowering=False, debug=False)
    ut_d = nc.dram_tensor("ut", [KP, n_per], dt.float32r, kind="ExternalInput").ap()
    vt_d = nc.dram_tensor("vt", [KP, m_tot], dt.float32r, kind="ExternalInput").ap()
    out_d = nc.dram_tensor("out", [n_per, m_tot], dt.float32, kind="ExternalOutput").ap()

    MB = n_per // 128  # output strips per core
    CHUNK = 2048  # ACT granularity: 4 PSUM banks per activation op
    NCHUNK = m_tot // CHUNK
    JB = CHUNK // 512  # matmuls (PSUM banks) per chunk

    with tile.TileContext(nc) as tc:
        with (
            tc.tile_pool(name="const", bufs=1) as cpool,
            tc.tile_pool(name="psum", bufs=2, space="PSUM") as psum_pool,
            tc.tile_pool(name="strip", bufs=3) as strip_pool,
        ):
            ut_s = cpool.tile([KP, n_per], dt.float32r, tag="ut")
            nc.sync.dma_start(ut_s[:], ut_d[:])
            # two-chunk load so the first matmuls start after ~2 MB
            vt_s = cpool.tile([KP, m_tot], dt.float32r, tag="vt")
            for c in range(2):
                csl = slice(c * (m_tot // 2), (c + 1) * (m_tot // 2))
                nc.sync.dma_start(vt_s[:, csl], vt_d[:, csl])

            for m in range(MB):
                strip = strip_pool.tile([128, m_tot], dt.float32)
                msl = slice(m * 128, (m + 1) * 128)
                for c in range(NCHUNK):
                    csl = slice(c * CHUNK, (c + 1) * CHUNK)
                    ps = psum_pool.tile([128, CHUNK], dt.float32)
                    for j in range(JB):
                        jsl = slice(j * 512, (j + 1) * 512)
                        vsl = slice(c * CHUNK + j * 512, c * CHUNK + (j + 1) * 512)
                        nc.tensor.matmul(ps[:, jsl], ut_s[:, msl], vt_s[:, vsl])
                    nc.scalar.activation(
                        strip[:, csl],
                        ps[:],
                        mybir.ActivationFunctionType.Exp,
                        scale=-gamma,
                    )
                    # store every 2 MB (2 chunks) as soon as the exps are
                    # done: keeps the DMA engines draining continuously while
                    # using 16 KB descriptors (fewer slices -> less profiler
                    # traffic on the shared AXI ports)
                    if c % 2 == 1:
                        osl = slice((c - 1) * CHUNK, (c + 1) * CHUNK)
                        nc.sync.dma_start(out_d[msl, osl], strip[:, osl])

    nc.compile()
    _BUILD_CACHE[key] = nc
    return nc


def _augment(x: np.ndarray, y: np.ndarray):
    """Host-side prep: transposed augmented operands, zero-padded to KP
    partitions (O(N*D) work)."""
    x = np.asarray(x, dtype=np.float32)
    y = np.asarray(y, dtype=np.float32)
    x2 = np.einsum("nd,nd->n", x, x).astype(np.float32)
    y2 = np.einsum("nd,nd->n", y, y).astype(np.float32)

    # Center the squared norms around their mean (E|x|^2 = D for unit-normal
    # data): the matmul then produces dist2 - 2D with small-magnitude
    # operands (better for the reduced-precision f32r path), and exp()'s
    # bias adds the -gamma*2D shift back.
    ut = np.zeros((KP, x.shape[0]), dtype=np.float32)
    ut[:D] = (-2.0 * x).T
    ut[D] = x2 - float(D)
    ut[D + 1] = 1.0
    ut[D + 2] = 1.0

    vt = np.zeros((KP, y.shape[0]), dtype=np.float32)
    vt[:D] = y.T
    vt[D] = 1.0
    vt[D + 1] = y2 - float(D)
    vt[D + 2] = 2.0 * float(D)
    return ut, vt


def kernel(x: np.ndarray, y: np.ndarray, gamma: np.ndarray) -> np.ndarray:
    global LAST_RESULTS
    from concourse.bass_utils import run_bass_kernel_spmd

    gamma_f = float(np.asarray(gamma).reshape(()))
    ut, vt = _augment(x, y)

    nc = _build(gamma_f, N_PER, N_Y)

    in_maps = []
    for c in range(N_CORES):
        in_maps.append(
            {
                "ut": np.ascontiguousarray(ut[:, c * N_PER : (c + 1) * N_PER]),
                "vt": vt,
            }
        )

    res = run_bass_kernel_spmd(nc, in_maps, core_ids=list(range(N_CORES)))
    LAST_RESULTS = res
    return np.concatenate([res.results[c]["out"] for c in range(N_CORES)], axis=0)
